# revision 3
# baseline (speedup 1.0000x reference)
"""MoE ExpertLayer kernel for Trainium2 (8 NeuronCores, data-parallel over tokens).

Reference computation (B=4, S=2048, D=1024, E=8):
    logits  = x @ W_router.T + b_router          # [B,S,E]
    probs   = softmax(logits, axis=-1)
    y_e     = x @ W_experts[e].T + b_experts[e]  # all experts, dense
    out     = sum_e probs[..., e] * y_e          # [B,S,D]

Sharding: data-parallel over the flattened token axis (8192 tokens -> 1024
tokens per core). Every core receives the full (transposed) expert weights and
computes its token shard end-to-end; no collectives are needed.

Per-core dataflow:
  - xT [D, T] resident in SBUF; expert weights streamed one expert at a time
    as WtT[e] = W_experts[e].T (so the contraction dim d lands on SBUF
    partitions for both matmul operands with contiguous DMA).
  - Router: 8 accumulating matmuls per token tile -> PSUM [128 tok, 8 e],
    + K=1 ones-matmul to add b_router; softmax via DVE reduce_max(negate) +
    ACT Exp(bias=-max, accum_out=sum) + DVE reciprocal + tensor_scalar_mul.
  - Bias fold: out bias term sum_e probs[t,e]*b_e[f] is a K=8 matmul
    (probs.T as stationary) accumulated straight into the output accumulator.
  - Experts: psum[t=128, f=512] accumulates 8 d-tile matmuls; the combine
    acc = psum * probs[:,e] + acc is one fused DVE scalar_tensor_tensor op.
"""

import os
import sys

for _p in ("/opt/trn_rl_repo", "/root/.axon_site/_ro/trn_rl_repo"):
    if os.path.isdir(_p) and _p not in sys.path:
        sys.path.insert(0, _p)

from contextlib import ExitStack

import ml_dtypes
import numpy as np

import concourse.bass as bass
import concourse.mybir as mybir
import concourse.tile as tile
from concourse import bacc
from concourse.bass import ts
from concourse.bass_utils import run_bass_kernel_spmd
from concourse.masks import make_identity

B, S, D, E = 4, 2048, 1024, 8
N_CORES = 8
T = B * S // N_CORES  # tokens per core = 1024
P = 128               # partitions
TT = T // P           # token tiles per core = 8
DT = D // P           # contraction tiles = 8
FN = 512              # matmul moving free dim (one PSUM bank of fp32)
FH = D // FN          # output column halves = 2

MODE = os.environ.get("KERNEL_MODE", "bf16")  # bf16 | f32r | f32


def _compute_dt(mode):
    return {
        "bf16": mybir.dt.bfloat16,
        "f32r": mybir.dt.float32r,
        "f32": mybir.dt.float32,
    }[mode]


def _np_dt(mode):
    return {"bf16": ml_dtypes.bfloat16, "f32r": np.float32, "f32": np.float32}[mode]


def build(mode=MODE):
    """Build the per-core Bass/Tile program (identical SPMD program on all cores)."""
    cdt = _compute_dt(mode)
    f32 = mybir.dt.float32

    nc = bacc.Bacc("TRN2", target_bir_lowering=False, debug=False)

    xT_d = nc.dram_tensor("xT", [D, T], cdt, kind="ExternalInput").ap()
    Wt_d = nc.dram_tensor("Wt", [E, D, D], cdt, kind="ExternalInput").ap()
    be_d = nc.dram_tensor("be", [E, D], cdt, kind="ExternalInput").ap()
    WrT_d = nc.dram_tensor("WrT", [D, E], cdt, kind="ExternalInput").ap()
    br_d = nc.dram_tensor("br", [1, E], cdt, kind="ExternalInput").ap()
    out_d = nc.dram_tensor("out", [T, D], f32, kind="ExternalOutput").ap()

    with tile.TileContext(nc) as tc, ExitStack() as ctx:
        singles = ctx.enter_context(tc.tile_pool(name="singles", bufs=1))
        wpool = ctx.enter_context(tc.tile_pool(name="wpool", bufs=3))
        small = ctx.enter_context(tc.tile_pool(name="small", bufs=4))
        ppool = ctx.enter_context(tc.tile_pool(name="psum_e", bufs=3, space="PSUM"))
        pbias = ctx.enter_context(tc.tile_pool(name="psum_b", bufs=2, space="PSUM"))
        prout = ctx.enter_context(tc.tile_pool(name="psum_r", bufs=1, space="PSUM"))

        # Resident tensors
        xT = singles.tile([P, DT, T], cdt)
        nc.sync.dma_start(out=xT, in_=xT_d.rearrange("(dt p) t -> p dt t", p=P))
        WrT = singles.tile([P, DT, E], cdt)
        nc.sync.dma_start(out=WrT, in_=WrT_d.rearrange("(dt p) e -> p dt e", p=P))
        be = singles.tile([E, D], cdt)
        nc.sync.dma_start(out=be, in_=be_d)
        br = singles.tile([1, E], cdt)
        nc.sync.dma_start(out=br, in_=br_d)
        ones = singles.tile([1, P], cdt)
        nc.vector.memset(ones, 1.0)
        ident = singles.tile([P, P], f32)
        make_identity(nc, ident)

        acc = singles.tile([P, TT, D], f32)
        probs = singles.tile([P, TT, E], f32)
        probsT = singles.tile([E, TT, P], cdt)

        # ---- Router: logits -> softmax -> probs / probs.T ----
        for tt in range(TT):
            pr = prout.tile([P, E], f32, tag="pr")
            for dt_ in range(DT):
                nc.tensor.matmul(
                    pr, xT[:, dt_, ts(tt, P)], WrT[:, dt_, :],
                    start=(dt_ == 0), stop=False,
                )
            # += b_router broadcast over the 128 tokens (K=1 rank-1 matmul)
            nc.tensor.matmul(pr, ones, br, start=False, stop=True)

            negmax = small.tile([P, 1], f32, tag="negmax")
            nc.vector.reduce_max(
                out=negmax, in_=pr, axis=mybir.AxisListType.X, negate=True
            )
            z = small.tile([P, E], f32, tag="z")
            ssum = small.tile([P, 1], f32, tag="ssum")
            nc.scalar.activation(
                out=z, in_=pr, func=mybir.ActivationFunctionType.Exp,
                bias=negmax, scale=1.0, accum_out=ssum,
            )
            rec = small.tile([P, 1], f32, tag="rec")
            nc.vector.reciprocal(rec, ssum)
            nc.vector.tensor_scalar_mul(probs[:, tt, :], z, rec)

            # probs.T for the bias-fold matmul: [128 tok, 8 e] -> [8 e, 128 tok]
            pT = prout.tile([E, P], f32, tag="pT")
            nc.tensor.transpose(pT, probs[:, tt, :], ident)
            nc.vector.tensor_copy(probsT[:, tt, :], pT)

        # ---- Bias fold: acc[t, f] = sum_e probs[t, e] * b_experts[e, f] ----
        for tt in range(TT):
            for fh in range(FH):
                pb = pbias.tile([P, FN], f32, tag="pb")
                nc.tensor.matmul(
                    pb, probsT[:, tt, :], be[:, ts(fh, FN)], start=True, stop=True
                )
                nc.vector.tensor_copy(acc[:, tt, ts(fh, FN)], pb)

        # ---- Experts: stream W, accumulate weighted outputs ----
        for e in range(E):
            w = wpool.tile([P, DT, D], cdt, tag="w")
            nc.sync.dma_start(
                out=w, in_=Wt_d[e].rearrange("(dt p) f -> p dt f", p=P)
            )
            for tt in range(TT):
                for fh in range(FH):
                    pe_ = ppool.tile([P, FN], f32, tag="pe")
                    for dt_ in range(DT):
                        nc.tensor.matmul(
                            pe_, xT[:, dt_, ts(tt, P)], w[:, dt_, ts(fh, FN)],
                            start=(dt_ == 0), stop=(dt_ == DT - 1),
                        )
                    # acc = psum * probs[:, e] + acc  (one fused DVE op)
                    nc.vector.scalar_tensor_tensor(
                        out=acc[:, tt, ts(fh, FN)],
                        in0=pe_,
                        scalar=probs[:, tt, e : e + 1],
                        in1=acc[:, tt, ts(fh, FN)],
                        op0=mybir.AluOpType.mult,
                        op1=mybir.AluOpType.add,
                    )

        # ---- Store ----
        nc.sync.dma_start(out=out_d.rearrange("(tt p) f -> p tt f", p=P), in_=acc)

    nc.compile()
    return nc


def prep_inputs(x, W_experts, b_experts, W_router, b_router, mode=MODE):
    """Host-side marshalling: shard tokens, transpose so the contraction dim
    is DMA-contiguous onto SBUF partitions, cast to the compute dtype."""
    ndt = _np_dt(mode)
    x = np.asarray(x, dtype=np.float32).reshape(B * S, D)
    Wt = np.ascontiguousarray(
        np.asarray(W_experts, dtype=np.float32).transpose(0, 2, 1)
    ).astype(ndt)  # [E, D_in, D_out]
    WrT = np.ascontiguousarray(np.asarray(W_router, dtype=np.float32).T).astype(ndt)
    be = np.asarray(b_experts, dtype=np.float32).astype(ndt)
    br = np.asarray(b_router, dtype=np.float32).reshape(1, E).astype(ndt)
    in_maps = []
    for c in range(N_CORES):
        xT = np.ascontiguousarray(x[c * T : (c + 1) * T, :].T).astype(ndt)
        in_maps.append({"xT": xT, "Wt": Wt, "be": be, "WrT": WrT, "br": br})
    return in_maps


_BUILT = {}


def get_built(mode=MODE):
    if mode not in _BUILT:
        _BUILT[mode] = build(mode)
    return _BUILT[mode]


def wait_device_ready(max_tries=8, sleep_s=20):
    """Poke the axon-tunneled devices until they respond. A crashed prior
    process can leave the remote exec unit wedged for a minute or two;
    the terminal recycles it on subsequent connection attempts."""
    import time

    import jax
    import jax.numpy as jnp

    for attempt in range(max_tries):
        try:
            devs = jax.devices()
            for d in devs[:1]:
                a = jax.device_put(jnp.ones((2, 2)), d)
                np.asarray(a)
            return True
        except Exception as exc:  # noqa: BLE001
            if attempt == max_tries - 1:
                raise
            print(f"device not ready (attempt {attempt + 1}): {exc}; retrying")
            time.sleep(sleep_s)
    return False


def run_spmd(in_maps, mode=MODE, **kwargs):
    nc = get_built(mode)
    wait_device_ready()
    try:
        return run_bass_kernel_spmd(
            nc, in_maps, core_ids=list(range(N_CORES)), **kwargs
        )
    except Exception as exc:  # noqa: BLE001
        print(f"run_bass_kernel_spmd failed ({exc}); retrying once after re-poke")
        wait_device_ready()
        return run_bass_kernel_spmd(
            nc, in_maps, core_ids=list(range(N_CORES)), **kwargs
        )


def kernel(x, W_experts, b_experts, W_router, b_router):
    in_maps = prep_inputs(x, W_experts, b_experts, W_router, b_router)
    res = run_spmd(in_maps)
    out = np.concatenate(
        [np.asarray(res.results[c]["out"], dtype=np.float32) for c in range(N_CORES)],
        axis=0,
    )
    return out.reshape(B, S, D)


# revision 5
# speedup vs baseline: 1.0592x; 1.0592x over previous
"""MoE ExpertLayer kernel for Trainium2 (8 NeuronCores, data-parallel over tokens).

Reference computation (B=4, S=2048, D=1024, E=8):
    logits  = x @ W_router.T + b_router          # [B,S,E]
    probs   = softmax(logits, axis=-1)
    y_e     = x @ W_experts[e].T + b_experts[e]  # all experts, dense
    out     = sum_e probs[..., e] * y_e          # [B,S,D]

Sharding: data-parallel over the flattened token axis (8192 tokens -> 1024
tokens per core). Every core receives the full (transposed) expert weights and
computes its token shard end-to-end; no collectives are needed.

Per-core dataflow:
  - xT [D, T] resident in SBUF; expert weights streamed one expert at a time
    as WtT[e] = W_experts[e].T (so the contraction dim d lands on SBUF
    partitions for both matmul operands with contiguous DMA).
  - Router: 8 accumulating matmuls per token tile -> PSUM [128 tok, 8 e],
    + K=1 ones-matmul to add b_router; softmax via DVE reduce_max(negate) +
    ACT Exp(bias=-max, accum_out=sum) + DVE reciprocal + tensor_scalar_mul.
  - Bias fold: out bias term sum_e probs[t,e]*b_e[f] is a K=8 matmul
    (probs.T as stationary) accumulated straight into the output accumulator.
  - Experts: psum[t=128, f=512] accumulates 8 d-tile matmuls; the combine
    acc = psum * probs[:,e] + acc is one fused DVE scalar_tensor_tensor op.
"""

import os
import sys

for _p in ("/opt/trn_rl_repo", "/root/.axon_site/_ro/trn_rl_repo"):
    if os.path.isdir(_p) and _p not in sys.path:
        sys.path.insert(0, _p)

from contextlib import ExitStack

import ml_dtypes
import numpy as np

import concourse.bass as bass
import concourse.mybir as mybir
import concourse.tile as tile
from concourse import bacc
from concourse.bass import ts
from concourse.bass_utils import run_bass_kernel_spmd
from concourse.masks import make_identity

B, S, D, E = 4, 2048, 1024, 8
N_CORES = 8
T = B * S // N_CORES  # tokens per core = 1024
P = 128               # partitions
TT = T // P           # token tiles per core = 8
DT = D // P           # contraction tiles = 8
FN = 512              # matmul moving free dim (one PSUM bank of fp32)
FH = D // FN          # output column halves = 2

MODE = os.environ.get("KERNEL_MODE", "bf16")  # bf16 | f32r | f32


def _compute_dt(mode):
    return {
        "bf16": mybir.dt.bfloat16,
        "f32r": mybir.dt.float32r,
        "f32": mybir.dt.float32,
    }[mode]


def _np_dt(mode):
    return {"bf16": ml_dtypes.bfloat16, "f32r": np.float32, "f32": np.float32}[mode]


def build(mode=MODE):
    """Build the per-core Bass/Tile program (identical SPMD program on all cores)."""
    cdt = _compute_dt(mode)
    f32 = mybir.dt.float32

    nc = bacc.Bacc("TRN2", target_bir_lowering=False, debug=False)

    xT_d = nc.dram_tensor("xT", [D, T], cdt, kind="ExternalInput").ap()
    Wt_d = nc.dram_tensor("Wt", [E, D, D], cdt, kind="ExternalInput").ap()
    be_d = nc.dram_tensor("be", [E, D], cdt, kind="ExternalInput").ap()
    WrT_d = nc.dram_tensor("WrT", [D, E], cdt, kind="ExternalInput").ap()
    br_d = nc.dram_tensor("br", [1, E], cdt, kind="ExternalInput").ap()
    out_d = nc.dram_tensor("out", [T, D], f32, kind="ExternalOutput").ap()

    with tile.TileContext(nc) as tc, ExitStack() as ctx:
        singles = ctx.enter_context(tc.tile_pool(name="singles", bufs=1))
        wpool = ctx.enter_context(tc.tile_pool(name="wpool", bufs=3))
        small = ctx.enter_context(tc.tile_pool(name="small", bufs=4))
        ppool = ctx.enter_context(tc.tile_pool(name="psum_e", bufs=2, space="PSUM"))
        pbias = ctx.enter_context(tc.tile_pool(name="psum_b", bufs=2, space="PSUM"))
        prout = ctx.enter_context(tc.tile_pool(name="psum_r", bufs=1, space="PSUM"))

        # Two HWDGE rings (sync=SP, scalar=ACT): spread big transfers across
        # both — a single ring saturates around ~120 GB/s for these patterns.
        hwdge = [nc.sync, nc.scalar]

        # Resident tensors. xT split in halves, one per ring, so the router
        # can start ~2x sooner; small tensors ride the gpsimd SWDGE path so
        # they don't queue behind the bulk loads.
        xT = singles.tile([P, DT, T], cdt)
        xT_src = xT_d.rearrange("(dt p) t -> p dt t", p=P)
        half = DT // 2
        nc.sync.dma_start(out=xT[:, :half, :], in_=xT_src[:, :half, :])
        nc.scalar.dma_start(out=xT[:, half:, :], in_=xT_src[:, half:, :])
        WrT = singles.tile([P, DT, E], cdt)
        nc.gpsimd.dma_start(out=WrT, in_=WrT_d.rearrange("(dt p) e -> p dt e", p=P))
        be = singles.tile([E, D], cdt)
        nc.gpsimd.dma_start(out=be, in_=be_d)
        br = singles.tile([1, E], cdt)
        nc.gpsimd.dma_start(out=br, in_=br_d)
        ones = singles.tile([1, P], cdt)
        nc.vector.memset(ones, 1.0)
        ident = singles.tile([P, P], f32)
        make_identity(nc, ident)

        acc = singles.tile([P, TT, D], f32)
        probs = singles.tile([P, TT, E], f32)
        probsT = singles.tile([E, TT, P], cdt)

        # ---- Router: logits -> softmax -> probs / probs.T ----
        for tt in range(TT):
            pr = prout.tile([P, E], f32, tag="pr")
            for dt_ in range(DT):
                nc.tensor.matmul(
                    pr, xT[:, dt_, ts(tt, P)], WrT[:, dt_, :],
                    start=(dt_ == 0), stop=False,
                )
            # += b_router broadcast over the 128 tokens (K=1 rank-1 matmul)
            nc.tensor.matmul(pr, ones, br, start=False, stop=True)

            negmax = small.tile([P, 1], f32, tag="negmax")
            nc.vector.reduce_max(
                out=negmax, in_=pr, axis=mybir.AxisListType.X, negate=True
            )
            z = small.tile([P, E], f32, tag="z")
            ssum = small.tile([P, 1], f32, tag="ssum")
            nc.scalar.activation(
                out=z, in_=pr, func=mybir.ActivationFunctionType.Exp,
                bias=negmax, scale=1.0, accum_out=ssum,
            )
            rec = small.tile([P, 1], f32, tag="rec")
            nc.vector.reciprocal(rec, ssum)
            nc.vector.tensor_scalar_mul(probs[:, tt, :], z, rec)

            # probs.T for the bias-fold matmul: [128 tok, 8 e] -> [8 e, 128 tok]
            pT = prout.tile([E, P], f32, tag="pT")
            nc.tensor.transpose(pT, probs[:, tt, :], ident)
            nc.vector.tensor_copy(probsT[:, tt, :], pT)

        # ---- Bias fold: acc[t, f] = sum_e probs[t, e] * b_experts[e, f] ----
        for tt in range(TT):
            for fh in range(FH):
                pb = pbias.tile([P, FN], f32, tag="pb")
                nc.tensor.matmul(
                    pb, probsT[:, tt, :], be[:, ts(fh, FN)], start=True, stop=True
                )
                nc.vector.tensor_copy(acc[:, tt, ts(fh, FN)], pb)

        # ---- Experts: stream W, accumulate weighted outputs ----
        out_dst = out_d.rearrange("(tt p) f -> p tt f", p=P)
        for e in range(E):
            w = wpool.tile([P, DT, D], cdt, tag="w")
            w_src = Wt_d[e].rearrange("(dt p) f -> p dt f", p=P)
            # split each expert's 2MB across both HWDGE rings
            nc.sync.dma_start(out=w[:, :half, :], in_=w_src[:, :half, :])
            nc.scalar.dma_start(out=w[:, half:, :], in_=w_src[:, half:, :])
            for tt in range(TT):
                # one stationary load serves both output halves: accumulate
                # the fh=0 and fh=1 PSUM groups side by side per d-tile
                pe0 = ppool.tile([P, FN], f32, tag="pe0")
                pe1 = ppool.tile([P, FN], f32, tag="pe1")
                for dt_ in range(DT):
                    lhsT = xT[:, dt_, ts(tt, P)]
                    st = dt_ == 0
                    sp = dt_ == DT - 1
                    nc.tensor.matmul(
                        pe0, lhsT, w[:, dt_, 0:FN], start=st, stop=sp
                    )
                    nc.tensor.matmul(
                        pe1, lhsT, w[:, dt_, FN : 2 * FN], start=st, stop=sp
                    )
                for fh, pe_ in ((0, pe0), (1, pe1)):
                    # acc = psum * probs[:, e] + acc  (one fused DVE op)
                    nc.vector.scalar_tensor_tensor(
                        out=acc[:, tt, ts(fh, FN)],
                        in0=pe_,
                        scalar=probs[:, tt, e : e + 1],
                        in1=acc[:, tt, ts(fh, FN)],
                        op0=mybir.AluOpType.mult,
                        op1=mybir.AluOpType.add,
                    )
                if e == E - 1:
                    # final expert: stream each finished token tile out now so
                    # the store overlaps the remaining compute
                    hwdge[tt % 2].dma_start(
                        out=out_dst[:, tt, :], in_=acc[:, tt, :]
                    )

    nc.compile()
    return nc


def prep_inputs(x, W_experts, b_experts, W_router, b_router, mode=MODE):
    """Host-side marshalling: shard tokens, transpose so the contraction dim
    is DMA-contiguous onto SBUF partitions, cast to the compute dtype."""
    ndt = _np_dt(mode)
    x = np.asarray(x, dtype=np.float32).reshape(B * S, D)
    Wt = np.ascontiguousarray(
        np.asarray(W_experts, dtype=np.float32).transpose(0, 2, 1)
    ).astype(ndt)  # [E, D_in, D_out]
    WrT = np.ascontiguousarray(np.asarray(W_router, dtype=np.float32).T).astype(ndt)
    be = np.asarray(b_experts, dtype=np.float32).astype(ndt)
    br = np.asarray(b_router, dtype=np.float32).reshape(1, E).astype(ndt)
    in_maps = []
    for c in range(N_CORES):
        xT = np.ascontiguousarray(x[c * T : (c + 1) * T, :].T).astype(ndt)
        in_maps.append({"xT": xT, "Wt": Wt, "be": be, "WrT": WrT, "br": br})
    return in_maps


_BUILT = {}


def get_built(mode=MODE):
    if mode not in _BUILT:
        _BUILT[mode] = build(mode)
    return _BUILT[mode]


def wait_device_ready(max_tries=8, sleep_s=20):
    """Poke the axon-tunneled devices until they respond. A crashed prior
    process can leave the remote exec unit wedged for a minute or two;
    the terminal recycles it on subsequent connection attempts."""
    import time

    import jax
    import jax.numpy as jnp

    for attempt in range(max_tries):
        try:
            devs = jax.devices()
            for d in devs[:1]:
                a = jax.device_put(jnp.ones((2, 2)), d)
                np.asarray(a)
            return True
        except Exception as exc:  # noqa: BLE001
            if attempt == max_tries - 1:
                raise
            print(f"device not ready (attempt {attempt + 1}): {exc}; retrying")
            time.sleep(sleep_s)
    return False


def run_spmd(in_maps, mode=MODE, **kwargs):
    nc = get_built(mode)
    wait_device_ready()
    try:
        return run_bass_kernel_spmd(
            nc, in_maps, core_ids=list(range(N_CORES)), **kwargs
        )
    except Exception as exc:  # noqa: BLE001
        print(f"run_bass_kernel_spmd failed ({exc}); retrying once after re-poke")
        wait_device_ready()
        return run_bass_kernel_spmd(
            nc, in_maps, core_ids=list(range(N_CORES)), **kwargs
        )


def kernel(x, W_experts, b_experts, W_router, b_router):
    in_maps = prep_inputs(x, W_experts, b_experts, W_router, b_router)
    res = run_spmd(in_maps)
    out = np.concatenate(
        [np.asarray(res.results[c]["out"], dtype=np.float32) for c in range(N_CORES)],
        axis=0,
    )
    return out.reshape(B, S, D)
